# revision 46
# baseline (speedup 1.0000x reference)
"""Causal self-attention (B=2, S=2048, D=1024, H=16) on 8 TRN2 NeuronCores.

Sharding: batch (2) x head-group (4 heads each) -> 8 cores. Each core computes
Q/K/V projections for its 4 heads, causal flash-attention, and a partial
output projection (its 256 columns of the concatenated head outputs against
the matching rows of Wo^T). Host sums the 4 partials per batch and adds the
bias terms (bv @ Wo.T + bo), which are x-independent.

All large inputs are packed host-side into ONE [128, 24576] fp16 tensor
(fp16 I/O halves HBM traffic vs f32; rel-err stays ~4e-4, far under the
2e-2 gate; fp8 was measured at 2.6e-2+ and rejected). Column map per
partition p:
  [     0:16384)  xT   s-major tiles: xt[p, sc, c, s] = x[b].T[128c+p, 512sc+s]
  [16384:18432)  wqT  tiles: wq[p, c, d] = Wq.T[:, sl][128c+p, d] (8 x 256)
  [18432:20480)  wkT  same for Wk
  [20480:22528)  wvT  same for Wv
  [22528:24576)  woT  tiles: wo[p, t, e] = Wo.T[sl, :][128t+p, e] (2 x 1024)
Biases travel in a tiny [128, 4] f32 side tensor (bq | bk halves).

DMA order per rep: x chunk 0, then Wq/Wk/Wv, biases, x chunks 1-3, and Wo
LAST in its own transfer — Wo's last reader is the final out-projection, so
a fused weight DMA would serialize rep i+1's whole input load behind rep
i's tail in the benchmark loop. The s-major x layout means one 1MB DMA
delivers everything proj(sc) needs.

Compute per core (all matmuls fp16 at 1 col/cycle, fp32 PSUM):
  - projections fp16; QT/KT evacuated to fp16 with bias fused (DVE)
  - scores^T[k,q] tiles via fp16 matmuls, 2 heads row-packed per 128
    partitions (concurrent in the PE array)
  - exp on ACT with the 1/sqrt(dk) scale fused, fp16 out
  - causal masking: multiplicative 0/1 fp16 mask on the 4 straddle shapes
  - PV matmul fp16 with a ones column appended to V so the softmax
    denominator falls out of the same matmul (psum row 64)
  - per-head normalize chains (copy denom row -> reciprocal_approx_fast ->
    gpsimd partition_broadcast -> multiply), head-ordered so the first
    o_ps PSUM buffer releases after one chain latency; op pool bufs=3 so
    the next pair's PV accumulation starts before both chains finish
  - out-projection fp16 against Wo^T rows; fp16 output DMA per 512-row chunk

The attention inner loop is ACT(exp)-throughput-bound while projections are
PE-bound, and the PE executes its queue strictly in order — so projection
and out-projection matmuls are emitted as generators of small quanta that
the attention k-loop drains between its own matmuls. V(j)'s projection is
drained INSIDE attn(j) (legal until its straddle tiles, which are the first
readers of v_sb[j]) — that keeps the late, filler-starved chunks fed.
Per-s-chunk SBUF tiles (not one big tensor) keep cross-phase dependencies
precise; PSUM budget: pv(2) + scores(3) + o(3) = 8 banks.
"""

import numpy as np

N_CORES = 8
B, S, D = 2, 2048, 1024
H_PER_CORE = 4
DSL = 256
NC_TILES = 8
SCH = 512
NSCH = S // SCH
NST = S // 128

XT_O = 0
WQ_O = 16384
WK_O = WQ_O + 2048
WV_O = WK_O + 2048
WO_O = WV_O + 2048
IN_COLS = WO_O + 2048  # 24576

_cache = {}


def _build(reps=1, dma="loop", drain=(1, 1, 1, 1), pools=(2, 3, 3), ep_bufs=8):
    import contextlib
    import concourse.mybir as mybir
    import concourse.tile as tile
    from concourse import bacc

    f32 = mybir.dt.float32
    f32r = mybir.dt.float32r
    f16 = mybir.dt.float16
    EXP = mybir.ActivationFunctionType.Exp

    nc = bacc.Bacc("TRN2", target_bir_lowering=False, debug=False,
                   num_devices=N_CORES)

    big = nc.dram_tensor("big", [128, IN_COLS], f16, kind="ExternalInput").ap()
    bqk = nc.dram_tensor("bqk", [128, 4], f32, kind="ExternalInput").ap()
    y = nc.dram_tensor("y", [S, D], f16, kind="ExternalOutput").ap()

    with tile.TileContext(nc) as tc:
        with contextlib.ExitStack() as ctx:
            singles = ctx.enter_context(tc.tile_pool(name="singles", bufs=1))
            work = ctx.enter_context(tc.tile_pool(name="work", bufs=1))

            big_sb = singles.tile([128, IN_COLS], f16)
            # x packed s-major: [sc, c, 512] so one 1MB DMA delivers
            # everything proj(sc) needs (QK + V of chunk sc).
            xt_sb = big_sb[:, XT_O:WQ_O].rearrange(
                "p (sc c s) -> p sc c s", sc=NSCH, c=NC_TILES)
            wq_sb = big_sb[:, WQ_O:WK_O].rearrange("p (c d) -> p c d", c=NC_TILES)
            wk_sb = big_sb[:, WK_O:WV_O].rearrange("p (c d) -> p c d", c=NC_TILES)
            wv_sb = big_sb[:, WV_O:WO_O].rearrange("p (c d) -> p c d", c=NC_TILES)
            wo_sb = big_sb[:, WO_O:IN_COLS].rearrange("p (t e) -> p t e", t=2)
            bqk_sb = singles.tile([128, 4], f32)

            # per-s-chunk tiles -> precise cross-phase dependencies
            qt_sb = [work.tile([128, 2, SCH], f16, name=f"qt{j}", tag=f"qt{j}")
                     for j in range(NSCH)]
            kt_sb = [work.tile([128, 2, SCH], f16, name=f"kt{j}", tag=f"kt{j}")
                     for j in range(NSCH)]
            v_sb = [work.tile([128, 4, 260], f16, name=f"v{j}", tag=f"v{j}")
                    for j in range(NSCH)]
            att_sb = [[work.tile([128, SCH], f16, name=f"att{j}_{p}", tag=f"att{j}_{p}")
                       for p in range(2)] for j in range(NSCH)]
            masks = [singles.tile([128, SCH], f16, name=f"mask{m}", tag=f"mask{m}")
                     for m in range(4)]

            # causal 0/1 masks: block row k (partition), col q;
            # valid iff q - k - 128*m >= 0
            for m in range(4):
                nc.gpsimd.memset(masks[m], 1.0)
                nc.gpsimd.affine_select(
                    out=masks[m], in_=masks[m],
                    compare_op=mybir.AluOpType.is_ge, fill=0.0,
                    base=-128 * m, pattern=[[1, SCH]], channel_multiplier=-1)
            # ones columns of V (col 64 of each head slot), written once:
            # per-rep V copies only touch cols 0..63 of each slot.
            for j in range(NSCH):
                nc.gpsimd.memset(v_sb[j], 1.0)

            def dma_in():
                # All INPUT loads ride the scalar HWDGE queue, all y stores
                # ride the sync queue: HWDGE queues are FIFO per issuing
                # engine, so mixing directions would park rep i+1's first
                # input load behind rep i's last y store. x s-chunk 0 +
                # Wq/Wk/Wv first (first matmuls need them); Wo last and
                # separate: its last reader is outp(3) at the very end of a
                # rep, so a fused weight DMA would serialize rep i+1's whole
                # input load behind rep i's tail.
                nc.scalar.dma_start(out=big_sb[:, XT_O:XT_O + 4096],
                                    in_=big[:, XT_O:XT_O + 4096])
                nc.scalar.dma_start(out=big_sb[:, WQ_O:WO_O], in_=big[:, WQ_O:WO_O])
                nc.scalar.dma_start(out=bqk_sb, in_=bqk)
                for sc in range(1, NSCH):
                    cs = slice(XT_O + 4096 * sc, XT_O + 4096 * (sc + 1))
                    nc.scalar.dma_start(out=big_sb[:, cs], in_=big[:, cs])
                nc.scalar.dma_start(out=big_sb[:, WO_O:IN_COLS], in_=big[:, WO_O:IN_COLS])

            if dma == "once":
                dma_in()

            def body(_iv=None):
                with contextlib.ExitStack() as bctx:
                    if dma == "loop":
                        dma_in()

                    pv = bctx.enter_context(tc.tile_pool(name="pv", bufs=pools[0], space="PSUM"))
                    sp_ = bctx.enter_context(tc.tile_pool(name="sp", bufs=pools[1], space="PSUM"))
                    op_ = bctx.enter_context(tc.tile_pool(name="op", bufs=pools[2], space="PSUM"))
                    ep = bctx.enter_context(tc.tile_pool(name="ep", bufs=ep_bufs))
                    bp = bctx.enter_context(tc.tile_pool(name="bp", bufs=4))
                    yo = bctx.enter_context(tc.tile_pool(name="yo", bufs=2))

                    def qk_gen(sc, halves=(0, 1)):
                        """Q/K projection for s-chunk sc as small PE quanta.

                        halves selects head-pair halves: attn(sc) pair p only
                        reads half p, so half 1 can be deferred into attn(sc)
                        pair 0's drain slots.
                        """
                        for half in halves:
                            for w_sb, dst, boff in ((wq_sb, qt_sb[sc], 0),
                                                    (wk_sb, kt_sb[sc], 2)):
                                ps = pv.tile([128, SCH], f32, name="pj", tag="pv")
                                for c in range(NC_TILES):
                                    nc.tensor.matmul(
                                        ps, lhsT=w_sb[:, c, 128 * half:128 * (half + 1)],
                                        rhs=xt_sb[:, sc, c, :],
                                        start=(c == 0), stop=(c == NC_TILES - 1))
                                    if c % 2:
                                        yield
                                nc.vector.tensor_scalar_add(
                                    dst[:, half, :], ps,
                                    bqk_sb[:, boff + half:boff + half + 1])
                                yield

                    def v_gen(sc):
                        """V projection for s-chunk sc as small PE quanta."""
                        for t4 in range(4):
                            v_ps = pv.tile([128, DSL], f32, name="vps", tag="pv")
                            for c in range(NC_TILES):
                                nc.tensor.matmul(
                                    v_ps, lhsT=xt_sb[:, sc, c, 128 * t4:128 * (t4 + 1)],
                                    rhs=wv_sb[:, c, :], start=(c == 0),
                                    stop=(c == NC_TILES - 1))
                                if c % 2:
                                    yield
                            nc.any.tensor_copy(
                                out=v_sb[sc].rearrange("p t (h e) -> p t h e", h=4)[:, t4, :, 0:64],
                                in_=v_ps.rearrange("p (h e) -> p h e", h=4))
                            yield

                    def outp_gen(j):
                        """Out-projection for q-chunk j as small PE quanta."""
                        y_sb = yo.tile([128, 4, D], f16, name="ysb", tag="ysb")
                        for t4 in range(4):
                            for e in range(2):
                                es = slice(512 * e, 512 * (e + 1))
                                y_ps = pv.tile([128, 512], f32, name="yps", tag="pv")
                                for pair in range(2):
                                    nc.tensor.matmul(
                                        y_ps, lhsT=att_sb[j][pair][:, 128 * t4:128 * (t4 + 1)],
                                        rhs=wo_sb[:, pair, es],
                                        start=(pair == 0), stop=(pair == 1))
                                if j == NSCH - 1:
                                    # rep tail: ACT is exp-idle here and DVE
                                    # is busy with the normalize chains
                                    nc.scalar.copy(out=y_sb[:, t4, es], in_=y_ps)
                                else:
                                    nc.any.tensor_copy(out=y_sb[:, t4, es], in_=y_ps)
                                yield
                        nc.sync.dma_start(
                            out=y[SCH * j:SCH * (j + 1), :].rearrange("(t p) e -> p t e", p=128),
                            in_=y_sb)
                        yield

                    def attn(j, bg, bg_early=None, early_rate=2, bg_p1=None):
                        # bg_early: quanta that must finish before the PV of
                        # tile 4j (V(j) work: this chunk's straddle tiles are
                        # its first readers). Paced at early_rate/tile, force-
                        # flushed when the straddle region starts. bg_p1:
                        # quanta only pair 1 depends on (its Q/K half) —
                        # drained during pair 0, flushed at the pair boundary.
                        T = 4 * (j + 1)
                        nd = drain[j]
                        early_left = bg_early
                        for pair in range(2):
                            if pair == 1 and bg_p1 is not None:
                                for _ in bg_p1:
                                    pass
                                bg_p1 = None
                            o_ps = [op_.tile([65, SCH], f32, name=f"ops{h}", tag="o")
                                    for h in range(2)]
                            prev = None

                            def emit_pv(exps, t, c0):
                                cs_ = slice(c0, SCH)
                                for h in range(2):
                                    hl = 2 * pair + h
                                    nc.tensor.matmul(
                                        o_ps[h][:, cs_], lhsT=v_sb[t // 4][:, t % 4, 65 * hl:65 * hl + 65],
                                        rhs=exps[h][:, cs_], start=(t == 0), stop=(t == T - 1))

                            for t in range(T):
                                m = t - 4 * j
                                # straddle tile m: columns < 128m are fully
                                # masked -> skip them in scores/exp/mask/PV
                                c0 = 128 * m if m > 0 else 0
                                cs_ = slice(c0, SCH)
                                s_ps = [sp_.tile([128, SCH], f32, name=f"sps{h}", tag="s")
                                        for h in range(2)]
                                for h in range(2):
                                    hp = slice(64 * h, 64 * (h + 1))
                                    nc.tensor.matmul(
                                        s_ps[h][:, cs_],
                                        lhsT=kt_sb[t // 4][hp, pair, 128 * (t % 4):128 * (t % 4 + 1)],
                                        rhs=qt_sb[j][hp, pair, cs_], start=True, stop=True)
                                exps = [ep.tile([128, SCH], f16, name=f"exps{h}", tag="e")
                                        for h in range(2)]
                                for h in range(2):
                                    nc.scalar.activation(out=exps[h][:, cs_], in_=s_ps[h][:, cs_],
                                                         func=EXP, scale=0.125)
                                if m >= 0:
                                    for h in range(2):
                                        nc.vector.tensor_mul(exps[h][:, cs_], exps[h][:, cs_],
                                                             masks[m][:, cs_])
                                if early_left is not None and m >= 0 and j > 0:
                                    # straddle region reached: flush V(j)
                                    for _ in early_left:
                                        pass
                                    early_left = None
                                if early_left is not None:
                                    for _ in range(early_rate):
                                        next(early_left, None)
                                if bg_p1 is not None:
                                    next(bg_p1, None)
                                for _ in range(nd):
                                    next(bg, None)
                                if prev is not None:
                                    emit_pv(*prev)
                                prev = (exps, t, c0)
                            emit_pv(*prev)

                            # normalize: att = O[0:64] * bcast(1/denom).
                            # Head-ordered chains so head 0's o_ps releases
                            # (and the next pair's PV unblocks, with op
                            # bufs=3) after ONE chain latency, not two.
                            for h in range(2):
                                bc = bp.tile([128, SCH], f32, name=f"bc{h}", tag="bc")
                                nc.vector.tensor_copy(out=bc[0:1, :], in_=o_ps[h][64:65, :])
                                nc.vector.reciprocal_approx_fast(
                                    out=bc[0:1, :], in_=bc[0:1, :])
                                nc.gpsimd.partition_broadcast(
                                    out_ap=bc[0:64, :], in_ap=bc[0:1, :])
                                nc.vector.tensor_mul(
                                    att_sb[j][pair][64 * h:64 * (h + 1), :],
                                    o_ps[h][0:64, :], bc[0:64, :])

                    def drain_all(bg):
                        for _ in bg:
                            pass

                    def chain(*gens):
                        for g in gens:
                            yield from g

                    # Only pair 0's Q/K half runs before attn(0); pair 1's
                    # half and V(0) drain inside attn(0) itself (V paced one
                    # group ahead of its PV). Later chunks drain V(j) early
                    # (their straddle tiles read it) plus outp(j-1) and
                    # QK(j+1) quanta between k-tiles — keeps PE fed in the
                    # late, filler-starved chunks and shrinks the serial
                    # prologue at each rep boundary.
                    drain_all(qk_gen(0, halves=(0,)))
                    for j in range(NSCH):
                        gens = []
                        if j > 0:
                            gens.append(outp_gen(j - 1))
                        if j + 1 < NSCH:
                            gens.append(qk_gen(j + 1))
                        bg = chain(*gens)
                        attn(j, bg, bg_early=v_gen(j),
                             early_rate=5 if j == 0 else 2,
                             bg_p1=qk_gen(0, halves=(1,)) if j == 0 else None)
                        drain_all(bg)
                    drain_all(outp_gen(NSCH - 1))

            if reps == 1:
                body()
            else:
                with tc.For_i(0, reps, 1):
                    body()

    nc.compile()
    return nc


def _get_nc(reps=1, **kw):
    key = (reps, tuple(sorted(kw.items())))
    if key not in _cache:
        _cache[key] = _build(reps, **kw)
    return _cache[key]


def _tiles(a, nt):
    # [nt*128, w] -> [128, nt*w] with [p, t*w:t*w+w] = a[128t+p, :]
    w = a.shape[1]
    return a.reshape(nt, 128, w).transpose(1, 0, 2).reshape(128, nt * w)


def make_in_maps(x, Wq, bq, Wk, bk, Wv, bv, Wo, bo):
    """Shard full inputs into 8 per-core input dicts (fp16 payload)."""
    in_maps = []
    for core in range(N_CORES):
        b, g = core // 4, core % 4
        sl = slice(DSL * g, DSL * (g + 1))
        # x s-major: cols [sc, c, s] with xt[p, sc, c, s] = x[b].T[128c+p, 512sc+s]
        xsm = x[b].T.reshape(8, 128, 4, 512).transpose(1, 2, 0, 3).reshape(128, 16384)
        big = np.concatenate([
            xsm,
            _tiles(np.ascontiguousarray(Wq[sl, :].T), 8),
            _tiles(np.ascontiguousarray(Wk[sl, :].T), 8),
            _tiles(np.ascontiguousarray(Wv[sl, :].T), 8),
            _tiles(np.ascontiguousarray(Wo[:, sl].T), 2),
        ], axis=1).astype(np.float16)
        bqk = np.concatenate([bq[sl].reshape(2, 128).T, bk[sl].reshape(2, 128).T],
                             axis=1)
        in_maps.append({"big": big, "bqk": np.ascontiguousarray(bqk)})
    return in_maps


def kernel(x, Wq, bq, Wk, bk, Wv, bv, Wo, bo):
    from concourse.bass_utils import run_bass_kernel_spmd

    x = np.asarray(x, dtype=np.float32)
    Wq, bq = np.asarray(Wq, np.float32), np.asarray(bq, np.float32)
    Wk, bk = np.asarray(Wk, np.float32), np.asarray(bk, np.float32)
    Wv, bv = np.asarray(Wv, np.float32), np.asarray(bv, np.float32)
    Wo, bo = np.asarray(Wo, np.float32), np.asarray(bo, np.float32)

    nc = _get_nc()
    in_maps = make_in_maps(x, Wq, bq, Wk, bk, Wv, bv, Wo, bo)
    res = run_bass_kernel_spmd(nc, in_maps, core_ids=list(range(N_CORES)))

    cvec = bv @ Wo.T + bo  # x-independent bias contribution
    out = np.zeros((B, S, D), dtype=np.float32)
    for core in range(N_CORES):
        out[core // 4] += res.results[core]["y"].astype(np.float32)
    out += cvec[None, None, :]
    return out

